# revision 1
# baseline (speedup 1.0000x reference)
"""Cross-modal attention kernel for Trainium2 -- data-parallel over batch on 8 cores.

Reference computation per sample (C=256, H=W=64, N=H*W=4096, dqk=32):
    q = Wq @ x + bq; k = Wk @ y + bk; v = Wv @ y + bv
    out = gamma * (v @ softmax_j(q^T k)^T) + x

Strategy (per core = one batch sample):
  - Projections run in float32r, attention in bf16/fp8 so PE matmuls stream
    at 1 cycle/row (fp32 would be 4).
  - Energy is computed TRANSPOSED (E^T[j,i], keys on partitions) so the
    attention-weighted sum contracts over the partition dim with no
    transposes.  exp() is applied unnormalized (logits are O(1) by
    construction: gain-0.02 weights), softmax normalization happens on the
    [C, IBLK] output instead of the [N, N] matrix.
  - The K=32 energy matmuls are 4-way row-packed (tile_position).
  - exp(E^T) and v^T are stored fp8e4m3; AV and the denominator both run as
    MatmulPerfMode.DoubleRow contractions (2 fp8 weights/PE cell), pairing
    consecutive j-tiles via 3D [K,2,N] APs.  The denominator is a DoubleRow
    ones-matmul accumulating sum_j exp(E^T)[j,i] in PSUM.
  - Software pipelining: AV for group g-2 issues after the energy matmuls of
    group g; block n's normalization tail is deferred into block n+1.

Differences from the bf16 version:
  - exp(E^T) and v^T are stored as fp8e4m3; the AV contraction runs in
    MatmulPerfMode.DoubleRow (2 fp8 weights per PE cell -> half the cycles),
    pairing consecutive j-tiles along the partition dim via 3D [K,2,N] APs.
  - The softmax denominator is ALSO a DoubleRow matmul: ones[128,2,128] as
    stationary -> den[i] accumulates sum_j exp(E^T)[j,i] in PSUM, which
    removes the whole DVE accumulate+fold chain of the bf16 version.
  - gamma is applied as a per-partition tensor_scalar multiply on 1/den.
"""

import sys

if "/opt/trn_rl_repo" not in sys.path:
    sys.path.insert(0, "/opt/trn_rl_repo")

import numpy as np

import concourse.bacc as bacc
import concourse.mybir as mybir
import concourse.tile as tile
from concourse.bass_utils import run_bass_kernel_spmd

F32 = mybir.dt.float32
F32R = mybir.dt.float32r
BF16 = mybir.dt.bfloat16
FP8 = mybir.dt.float8e4

B, C, HW, D = 8, 256, 4096, 32
CH = C // 128
IBLK = 512
NIB = HW // IBLK
NJT = HW // 128
NPAIR = NJT // 2
EXPF = mybir.ActivationFunctionType.Exp
MULT = mybir.AluOpType.mult
ADD = mybir.AluOpType.add
DROW = mybir.MatmulPerfMode.DoubleRow


def _build():
    nc = bacc.Bacc("TRN2", target_bir_lowering=False, debug=False, num_devices=8)

    xr = nc.dram_tensor("xr", [C, HW], F32R, kind="ExternalInput")
    xf = nc.dram_tensor("xf", [C, HW], F32, kind="ExternalInput")
    yr = nc.dram_tensor("yr", [C, HW], F32R, kind="ExternalInput")
    wqT = nc.dram_tensor("wqT", [C, D], F32R, kind="ExternalInput")
    wkT = nc.dram_tensor("wkT", [C, D], F32R, kind="ExternalInput")
    wvT = nc.dram_tensor("wvT", [C, C], F32R, kind="ExternalInput")
    bqd = nc.dram_tensor("bqd", [D, 1], F32, kind="ExternalInput")
    bkd = nc.dram_tensor("bkd", [D, 1], F32, kind="ExternalInput")
    gbvd = nc.dram_tensor("gbvd", [128, CH], F32, kind="ExternalInput")
    gmd = nc.dram_tensor("gmd", [128, 1], F32, kind="ExternalInput")
    out = nc.dram_tensor("out", [C, HW], F32, kind="ExternalOutput")

    tc = tile.TileContext(nc)
    with tc:
        with (
            tc.tile_pool(name="cst", bufs=1) as cst,
            tc.tile_pool(name="qkv", bufs=1) as qkv,
        ):
            wq_sb = cst.tile([128, CH * D], F32R)
            wk_sb = cst.tile([128, CH * D], F32R)
            wv_sb = cst.tile([128, CH * C], F32R)
            bq_sb = cst.tile([D, 1], F32)
            bk_sb = cst.tile([D, 1], F32)
            gbv_sb = cst.tile([128, CH], F32)
            gm_sb = cst.tile([128, 1], F32)
            ones_sb = cst.tile([128, 2 * 128], FP8)
            nc.vector.memset(ones_sb[:], 1.0)
            nc.gpsimd.dma_start(bq_sb[:], bqd[:])
            nc.gpsimd.dma_start(bk_sb[:], bkd[:])
            nc.gpsimd.dma_start(gbv_sb[:], gbvd[:])
            nc.gpsimd.dma_start(gm_sb[:], gmd[:])

            q4 = qkv.tile([128, HW], BF16)
            k4 = qkv.tile([128, HW], BF16)
            vt = qkv.tile([128, NJT * C], FP8)

            NG = NJT // 4
            ptp = None  # assigned when the phase-B pools open
            psE = None

            def et_group(n, g, pt):
                # energy for (i-block n, group g): 4 row-packed K=32 matmuls
                # into two 2-bank psum tiles, then exp into pt (fp8)
                ets = [
                    psE.tile([128, 2 * IBLK], F32,
                             name=f"et{h}_{n}_{g}", tag="et", bufs=2)
                    for h in range(2)
                ]
                for q in range(4):
                    jt = 4 * g + q
                    nc.tensor.matmul(
                        ets[q // 2][:, (q % 2) * IBLK:(q % 2 + 1) * IBLK],
                        k4[32 * q:32 * (q + 1), jt * 128:(jt + 1) * 128],
                        q4[32 * q:32 * (q + 1), n * IBLK:(n + 1) * IBLK],
                        start=True,
                        stop=True,
                        tile_position=(32 * q, 0),
                    )
                for h in range(2):
                    nc.scalar.activation(
                        pt[:, (4 * g + 2 * h) * IBLK:(4 * g + 2 * h + 2) * IBLK],
                        ets[h][:], EXPF,
                    )

            with (
                tc.tile_pool(name="xy", bufs=1) as xy,
                tc.tile_pool(name="psA", bufs=4, space="PSUM") as psA,
            ):
                xr_sb = xy.tile([128, CH * HW], F32R)
                yr_sb = xy.tile([128, CH * HW], F32R)

                def in_chunk(src, dst_sb, h, c0, c1):
                    nc.sync.dma_start(
                        dst_sb[:, h * HW + c0: h * HW + c1],
                        src[h * 128:(h + 1) * 128, c0:c1],
                    )

                for h in range(CH):
                    nc.sync.dma_start(wq_sb[:, h * D:(h + 1) * D], wqT[h * 128:(h + 1) * 128, :])
                for h in range(CH):
                    in_chunk(xr, xr_sb, h, 0, IBLK)
                for h in range(CH):
                    nc.sync.dma_start(wk_sb[:, h * D:(h + 1) * D], wkT[h * 128:(h + 1) * 128, :])
                for h in range(CH):
                    in_chunk(yr, yr_sb, h, 0, IBLK)
                for h in range(CH):
                    nc.sync.dma_start(wv_sb[:, h * C:(h + 1) * C], wvT[h * 128:(h + 1) * 128, :])
                for ic in range(1, NIB):
                    c0, c1 = ic * IBLK, (ic + 1) * IBLK
                    for h in range(CH):
                        in_chunk(xr, xr_sb, h, c0, c1)
                        in_chunk(yr, yr_sb, h, c0, c1)
                for ic in range(NIB):
                    c0, c1 = ic * IBLK, (ic + 1) * IBLK
                    for w_sb, b_sb, src, dst in (
                        (wq_sb, bq_sb, xr_sb, q4),
                        (wk_sb, bk_sb, yr_sb, k4),
                    ):
                        ps = psA.tile([D, IBLK], F32, name=f"qk_{ic}", tag="qk_ps")
                        for h in range(CH):
                            nc.tensor.matmul(
                                ps[:],
                                w_sb[:, h * D:(h + 1) * D],
                                src[:, h * HW + c0: h * HW + c1],
                                start=(h == 0),
                                stop=(h == CH - 1),
                            )
                        nc.vector.tensor_scalar_add(
                            dst[0:D, c0:c1], ps[:], b_sb[:, 0:1]
                        )
                        for g in range(1, 4):
                            nc.gpsimd.dma_start(
                                dst[32 * g:32 * (g + 1), c0:c1], dst[0:D, c0:c1]
                            )
                    for jt in range(4 * ic, 4 * ic + 4):
                        ps = psA.tile([128, C], F32, name=f"vt_{jt}", tag="vt_ps")
                        for h in range(CH):
                            nc.tensor.matmul(
                                ps[:],
                                yr_sb[:, h * HW + jt * 128: h * HW + (jt + 1) * 128],
                                wv_sb[:, h * C:(h + 1) * C],
                                start=(h == 0),
                                stop=(h == CH - 1),
                            )
                        nc.vector.tensor_copy(vt[:, jt * C:(jt + 1) * C], ps[:])

            with (
                tc.tile_pool(name="ptp", bufs=2) as ptp,
                tc.tile_pool(name="wrk", bufs=2) as wrk,
                tc.tile_pool(name="psE", bufs=1, space="PSUM") as psE,
                tc.tile_pool(name="psAV", bufs=1, space="PSUM") as psAV,
            ):
                def make_tail(n, av, den):
                    def tail():
                        rgb = wrk.tile([128, IBLK], F32, name=f"rgb_{n}", tag="rgb")
                        nc.vector.reciprocal(rgb[:], den[:])
                        rgbg = wrk.tile([128, IBLK], F32, name=f"rgbg_{n}", tag="rgbg")
                        nc.vector.tensor_scalar(
                            rgbg[:], rgb[:], gm_sb[:, 0:1], None, MULT
                        )
                        for ch in range(CH):
                            xf_t = wrk.tile([128, IBLK], F32,
                                            name=f"xf_{n}_{ch}", tag="xf")
                            nc.sync.dma_start(
                                xf_t[:],
                                xf[ch * 128:(ch + 1) * 128, n * IBLK:(n + 1) * IBLK],
                            )
                            tmp = wrk.tile([128, IBLK], F32,
                                           name=f"tmp_{n}_{ch}", tag="tmp")
                            nc.vector.tensor_tensor(tmp[:], av[ch][:], rgbg[:], MULT)
                            ot = wrk.tile([128, IBLK], F32, name=f"ot_{n}_{ch}", tag="ot")
                            nc.vector.scalar_tensor_tensor(
                                ot[:], tmp[:], gbv_sb[:, ch:ch + 1], xf_t[:], ADD, ADD
                            )
                            nc.sync.dma_start(
                                out[ch * 128:(ch + 1) * 128, n * IBLK:(n + 1) * IBLK],
                                ot[:],
                            )
                    return tail

                ones_pair = ones_sb[:].rearrange("P (s c) -> P s c", s=2)

                pending_tail = None
                for n in range(NIB):
                    pt = ptp.tile([128, NJT * IBLK], FP8, name=f"pt_{n}", tag="pt")
                    av = [
                        psAV.tile([128, IBLK], F32, name=f"av{ch}_{n}", tag=f"av{ch}")
                        for ch in range(CH)
                    ]
                    den = psAV.tile([128, IBLK], F32, name=f"den_{n}", tag="den")

                    def av_pairs(g, pt=pt, av=av, den=den, n=n):
                        # DoubleRow AV + denominator for the 2 j-tile pairs of
                        # group g: virtual K=256 contracts two j-tiles at once
                        for p in (2 * g, 2 * g + 1):
                            ptp_ap = pt[:, 2 * p * IBLK:(2 * p + 2) * IBLK].rearrange(
                                "P (s N) -> P s N", s=2
                            )
                            vtp_ap = vt[:, 2 * p * C:(2 * p + 2) * C].rearrange(
                                "P (s c) -> P s c", s=2
                            )
                            for ch in range(CH):
                                nc.tensor.matmul(
                                    av[ch][:],
                                    vtp_ap[:, :, ch * 128:(ch + 1) * 128],
                                    ptp_ap,
                                    start=(p == 0),
                                    stop=(p == NPAIR - 1),
                                    perf_mode=DROW,
                                    skip_group_check=True,
                                )
                            nc.tensor.matmul(
                                den[:],
                                ones_pair,
                                ptp_ap,
                                start=(p == 0),
                                stop=(p == NPAIR - 1),
                                perf_mode=DROW,
                                skip_group_check=True,
                            )

                    for g in range(NG):
                        et_group(n, g, pt)
                        if g == 0 and pending_tail is not None:
                            pending_tail()
                            pending_tail = None
                        if g >= 2:
                            av_pairs(g - 2)
                    av_pairs(NG - 2)
                    av_pairs(NG - 1)
                    pending_tail = make_tail(n, av, den)
                pending_tail()
    nc.compile()
    return nc


_NC_CACHE = {}


def kernel(x, y, Wq, bq, Wk, bk, Wv, bv, gamma):
    assert x.shape == (B, C, 64, 64)
    xs = np.ascontiguousarray(x.reshape(B, C, HW).astype(np.float32))
    ys = np.ascontiguousarray(y.reshape(B, C, HW).astype(np.float32))
    wqT = np.ascontiguousarray(Wq.T.astype(np.float32))
    wkT = np.ascontiguousarray(Wk.T.astype(np.float32))
    wvT = np.ascontiguousarray(Wv.T.astype(np.float32))
    bqh = np.ascontiguousarray(bq.astype(np.float32).reshape(D, 1))
    bkh = np.ascontiguousarray(bk.astype(np.float32).reshape(D, 1))
    g = float(np.asarray(gamma).reshape(-1)[0])
    gbvh = np.ascontiguousarray((g * bv.astype(np.float32)).reshape(CH, 128).T)
    gmh = np.full((128, 1), g, dtype=np.float32)

    if "nc" not in _NC_CACHE:
        _NC_CACHE["nc"] = _build()
    nc = _NC_CACHE["nc"]

    in_maps = [
        {
            "xr": xs[b], "xf": xs[b], "yr": ys[b],
            "wqT": wqT, "wkT": wkT, "wvT": wvT,
            "bqd": bqh, "bkd": bkh, "gbvd": gbvh, "gmd": gmh,
        }
        for b in range(B)
    ]
    res = run_bass_kernel_spmd(nc, in_maps, list(range(B)))
    outs = np.stack([res.results[b]["out"] for b in range(B)])
    return outs.reshape(B, C, 64, 64).astype(np.float32)



# revision 3
# speedup vs baseline: 1.0802x; 1.0802x over previous
"""Cross-modal attention kernel for Trainium2 -- data-parallel over batch on 8 cores.

Reference computation per sample (C=256, H=W=64, N=H*W=4096, dqk=32):
    q = Wq @ x + bq; k = Wk @ y + bk; v = Wv @ y + bv
    out = gamma * (v @ softmax_j(q^T k)^T) + x

The Activation engine is the roofline: exp() over the [N, N] energy matrix is
131072 columns at 0.8333 ns/col (~133 us including per-instruction access
overhead), so the whole schedule is built to keep exp() streaming with zero
gaps from ~4 us onward:

  - Projections are FUSED into the first two i-blocks instead of a separate
    phase: k and q chunks are projected just ahead of the energy matmuls of
    block 0 (the in-order PE queue never sits between phases), v during
    block 1.
  - Inputs are uploaded pre-quantized (x fp16, y fp8e4m3) so the ramp DMA is
    3 MB instead of 12 MB; the residual x is re-read in f32 during the steady
    state where DMA is otherwise idle.
  - Every attention matmul runs fp8 DoubleRow (0.5 cyc/row): energy uses
    q/k stored as [16, 2, N] (dqk=32 split across two PE weight rows), k/v
    projections contract y as [64, 2, *], AV and the softmax denominator pair
    consecutive j-tiles [128, 2, *].  PE busy ~75 us << Activation busy.
  - PSUM (8 banks) budget: et double-buffer 4 + kq ring 2 + pv ring 2 during
    blocks 0-1 (no den/av yet), then et 4 + den ring 2 + av 2.  The
    denominator for blocks 0/1 and AV for every block run as deferred 16-step
    DoubleRow bursts one block later (pt lives in a 3-deep SBUF ring); the
    last block's AV runs eagerly so the drain is only ~4 us.
  - Softmax normalization happens on the [C, IBLK] output (reciprocal of the
    ones-matmul denominator), never on the [N, N] matrix.
"""

import sys

if "/opt/trn_rl_repo" not in sys.path:
    sys.path.insert(0, "/opt/trn_rl_repo")

import ml_dtypes
import numpy as np

import concourse.bacc as bacc
import concourse.mybir as mybir
import concourse.tile as tile
from concourse.bass_utils import run_bass_kernel_spmd

F32 = mybir.dt.float32
F16 = mybir.dt.float16
F8 = mybir.dt.float8e4
F8NP = ml_dtypes.float8_e4m3

B, C, HW, D = 8, 256, 4096, 32
CH = C // 128
IBLK = 512
NIB = HW // IBLK          # 8 i-blocks
NJT = HW // 128           # 32 j-tiles
NPAIR = NJT // 2          # 16 DoubleRow pairs
NG = NJT // 4             # 8 energy groups per block (4 j-tiles each)
EXPF = mybir.ActivationFunctionType.Exp
MULT = mybir.AluOpType.mult
ADD = mybir.AluOpType.add
DROW = mybir.MatmulPerfMode.DoubleRow


def _build():
    nc = bacc.Bacc("TRN2", target_bir_lowering=False, debug=False, num_devices=8)

    x16d = nc.dram_tensor("x16d", [128, CH * HW], F16, kind="ExternalInput")
    y8d = nc.dram_tensor("y8d", [64, 4 * HW], F8, kind="ExternalInput")
    xfd = nc.dram_tensor("xfd", [C, HW], F32, kind="ExternalInput")
    wq16d = nc.dram_tensor("wq16d", [128, CH * D], F16, kind="ExternalInput")
    wk8d = nc.dram_tensor("wk8d", [64, 4 * D], F8, kind="ExternalInput")
    wv8d = nc.dram_tensor("wv8d", [64, 4 * C], F8, kind="ExternalInput")
    bqd = nc.dram_tensor("bqd", [D, 1], F32, kind="ExternalInput")
    bkd = nc.dram_tensor("bkd", [D, 1], F32, kind="ExternalInput")
    gbvd = nc.dram_tensor("gbvd", [128, CH], F32, kind="ExternalInput")
    gmd = nc.dram_tensor("gmd", [128, 1], F32, kind="ExternalInput")
    out = nc.dram_tensor("out", [C, HW], F32, kind="ExternalOutput")

    tc = tile.TileContext(nc)
    with tc:
        with (
            tc.tile_pool(name="cst", bufs=1) as cst,
            tc.tile_pool(name="ptp", bufs=3) as ptp,
            tc.tile_pool(name="qkb", bufs=4) as qkb,
            tc.tile_pool(name="wrk", bufs=2) as wrk,
            tc.tile_pool(name="psE", bufs=1, space="PSUM") as psE,
        ):
            wq_sb = cst.tile([128, CH * D], F16)
            wk_sb = cst.tile([64, 4 * D], F8)
            wv_sb = cst.tile([64, 4 * C], F8)
            bq_sb = cst.tile([D, 1], F32)
            bk_sb = cst.tile([D, 1], F32)
            gbv_sb = cst.tile([128, CH], F32)
            gm_sb = cst.tile([128, 1], F32)
            ones_sb = cst.tile([128, 2 * 128], F8)
            x_sb = cst.tile([128, CH * HW], F16)
            y_sb = cst.tile([64, 4 * HW], F8)
            q4f = cst.tile([16, 2 * HW], F8)
            k4f = cst.tile([16, 2 * HW], F8)
            vt = cst.tile([128, NJT * C], F8)

            nc.vector.memset(ones_sb[:], 1.0)
            nc.gpsimd.dma_start(bq_sb[:], bqd[:])
            nc.gpsimd.dma_start(bk_sb[:], bkd[:])
            nc.gpsimd.dma_start(gbv_sb[:], gbvd[:])
            nc.gpsimd.dma_start(gm_sb[:], gmd[:])

            # weights + inputs on the SP queue, y (k/v path) leading
            nc.sync.dma_start(wk_sb[:], wk8d[:])
            nc.sync.dma_start(wq_sb[:], wq16d[:])
            nc.sync.dma_start(wv_sb[:], wv8d[:])
            y4 = y_sb[:].rearrange("P (b N) -> P b N", b=4)
            x2 = x_sb[:].rearrange("P (h N) -> P h N", h=2)
            for g in range(NIB):
                nc.sync.dma_start(
                    y4[:, :, g * IBLK:(g + 1) * IBLK],
                    y8d[:, g * 4 * IBLK:(g + 1) * 4 * IBLK].rearrange(
                        "P (b N) -> P b N", b=4),
                )
                nc.sync.dma_start(
                    x2[:, :, g * IBLK:(g + 1) * IBLK],
                    x16d[:, g * 2 * IBLK:(g + 1) * 2 * IBLK].rearrange(
                        "P (h N) -> P h N", h=2),
                )

            q4r = q4f[:].rearrange("P (s N) -> P s N", s=2)
            k4r = k4f[:].rearrange("P (s N) -> P s N", s=2)
            wk4 = wk_sb[:].rearrange("P (b d) -> P b d", b=4)
            wv4 = wv_sb[:].rearrange("P (b c) -> P b c", b=4)
            ones_pair = ones_sb[:].rearrange("P (s c) -> P s c", s=2)

            def energy(n, g, pt):
                # 4 j-tiles of E^T[j, i-block n] as fp8 DoubleRow matmuls,
                # exp() into pt in two [128, 1024] activations
                c0, c1 = n * IBLK, (n + 1) * IBLK
                for h in range(2):
                    et = psE.tile([128, 2 * IBLK], F32,
                                  name=f"et{h}_{n}_{g}", tag="et", bufs=2)
                    for t in range(2):
                        jt = 4 * g + 2 * h + t
                        nc.tensor.matmul(
                            et[:, t * IBLK:(t + 1) * IBLK],
                            k4r[:, :, jt * 128:(jt + 1) * 128],
                            q4r[:, :, c0:c1],
                            start=True, stop=True,
                            perf_mode=DROW,
                        )
                    nc.scalar.activation(
                        pt[:, (4 * g + 2 * h) * IBLK:(4 * g + 2 * h + 2) * IBLK],
                        et[:], EXPF,
                    )

            def ptp_ap(pt, p):
                return pt[:, 2 * p * IBLK:(2 * p + 2) * IBLK].rearrange(
                    "P (s N) -> P s N", s=2)

            def den_pairs(pt, den, pairs):
                for p in pairs:
                    nc.tensor.matmul(
                        den[:], ones_pair, ptp_ap(pt, p),
                        start=(p == 0), stop=(p == NPAIR - 1),
                        perf_mode=DROW, skip_group_check=True,
                    )

            def av_pairs(pt, av, ch, pairs):
                for p in pairs:
                    nc.tensor.matmul(
                        av[:],
                        vt[:, 2 * p * C:(2 * p + 2) * C].rearrange(
                            "P (s c) -> P s c", s=2)[:, :, ch * 128:(ch + 1) * 128],
                        ptp_ap(pt, p),
                        start=(p == 0), stop=(p == NPAIR - 1),
                        perf_mode=DROW, skip_group_check=True,
                    )

            def den_tail(n, den):
                # rgbg = gamma / den, shared by both C-chunks of block n
                rgb = wrk.tile([128, IBLK], F32, name=f"rgb_{n}", tag="rgb")
                nc.vector.reciprocal(rgb[:], den[:])
                rgbg = wrk.tile([128, IBLK], F32, name=f"rgbg_{n}", tag="rgbg",
                                bufs=3)
                nc.vector.tensor_scalar(rgbg[:], rgb[:], gm_sb[:, 0:1], None, MULT)
                return rgbg

            def xf_fetch(n):
                xs = []
                for ch in range(CH):
                    xf_t = wrk.tile([128, IBLK], F32, name=f"xf_{n}_{ch}",
                                    tag="xf", bufs=6)
                    nc.gpsimd.dma_start(
                        xf_t[:],
                        xfd[ch * 128:(ch + 1) * 128, n * IBLK:(n + 1) * IBLK],
                    )
                    xs.append(xf_t)
                return xs

            def tail_ch(n, ch, av, rgbg, xf_t):
                tmp = wrk.tile([128, IBLK], F32, name=f"tmp_{n}_{ch}", tag="tmp")
                nc.vector.tensor_tensor(tmp[:], av[:], rgbg[:], MULT)
                ot = wrk.tile([128, IBLK], F32, name=f"ot_{n}_{ch}", tag="ot",
                              bufs=3)
                nc.vector.scalar_tensor_tensor(
                    ot[:], tmp[:], gbv_sb[:, ch:ch + 1], xf_t[:], ADD, ADD)
                nc.sync.dma_start(
                    out[ch * 128:(ch + 1) * 128, n * IBLK:(n + 1) * IBLK], ot[:])

            pts = {}
            dens = {}
            avs = {}
            rgbgs = {}
            xfs = {}

            def new_block(n):
                pts[n] = ptp.tile([128, NJT * IBLK], F8, name=f"pt_{n}", tag="pt")

            # ------------- blocks 0-1: projections + energy ---------------
            with tc.tile_pool(name="psP", bufs=1, space="PSUM") as psP:
                def k_proj(g):
                    c0, c1 = g * IBLK, (g + 1) * IBLK
                    ps = psP.tile([D, IBLK], F32, name=f"kps_{g}", tag="kq_ps",
                                  bufs=2)
                    for h in range(CH):
                        nc.tensor.matmul(
                            ps[:], wk4[:, 2 * h:2 * h + 2, :],
                            y4[:, 2 * h:2 * h + 2, c0:c1],
                            start=(h == 0), stop=(h == CH - 1), perf_mode=DROW)
                    kb = qkb.tile([D, IBLK], F8, name=f"kb_{g}", tag="qkb")
                    nc.vector.tensor_scalar_add(kb[:], ps[:], bk_sb[:, 0:1])
                    for s in range(2):
                        nc.gpsimd.dma_start(
                            k4f[0:16, s * HW + c0:s * HW + c1],
                            kb[16 * s:16 * (s + 1), :])

                def q_proj(g):
                    c0, c1 = g * IBLK, (g + 1) * IBLK
                    ps = psP.tile([D, IBLK], F32, name=f"qps_{g}", tag="kq_ps",
                                  bufs=2)
                    for h in range(CH):
                        nc.tensor.matmul(
                            ps[:], wq_sb[:, h * D:(h + 1) * D], x2[:, h, c0:c1],
                            start=(h == 0), stop=(h == CH - 1))
                    qb = qkb.tile([D, IBLK], F8, name=f"qb_{g}", tag="qkb")
                    nc.vector.tensor_scalar_add(qb[:], ps[:], bq_sb[:, 0:1])
                    for s in range(2):
                        nc.gpsimd.dma_start(
                            q4f[0:16, s * HW + c0:s * HW + c1],
                            qb[16 * s:16 * (s + 1), :])

                def v_proj(jt):
                    ps = psP.tile([128, 256], F32, name=f"vps_{jt}", tag="pv_ps",
                                  bufs=2)
                    for h in range(CH):
                        nc.tensor.matmul(
                            ps[:],
                            y4[:, 2 * h:2 * h + 2, jt * 128:(jt + 1) * 128],
                            wv4[:, 2 * h:2 * h + 2, :],
                            start=(h == 0), stop=(h == CH - 1), perf_mode=DROW)
                    nc.vector.tensor_copy(vt[:, jt * C:(jt + 1) * C], ps[:])

                # block 0: all k + q chunks, one k-chunk ahead of energy
                new_block(0)
                for g in range(NG):
                    if g == 0:
                        k_proj(0)
                        k_proj(1)
                        q_proj(0)
                        q_proj(1)
                    elif g <= NG - 2:
                        k_proj(g + 1)
                        q_proj(g + 1)
                    energy(0, g, pts[0])

                # block 1: energy + all v j-tiles
                new_block(1)
                for g in range(NG):
                    energy(1, g, pts[1])
                    for jt in (4 * g, 4 * g + 1, 4 * g + 2, 4 * g + 3):
                        v_proj(jt)
                xfs[0] = xf_fetch(0)

            # ------------- blocks 2..7 + deferred den/av ------------------
            with tc.tile_pool(name="psAV", bufs=1, space="PSUM") as psAV:
                def new_den(n):
                    dens[n] = psAV.tile([128, IBLK], F32, name=f"den_{n}",
                                        tag="den", bufs=2)

                def new_av(n, ch):
                    avs[(n, ch)] = psAV.tile([128, IBLK], F32,
                                             name=f"av{ch}_{n}",
                                             tag=f"av{ch}", bufs=1)

                def av_tail_full(m, ch):
                    new_av(m, ch)
                    av_pairs(pts[m], avs[(m, ch)], ch, range(NPAIR))
                    tail_ch(m, ch, avs[(m, ch)], rgbgs[m], xfs[m][ch])

                # block 2: den(0)/den(1) bursts, av(0), eager den(2)
                new_block(2)
                new_den(0)   # den ring order: 0 -> bufA, 2 -> bufB, 1 -> bufA
                new_den(2)
                for g in range(NG):
                    energy(2, g, pts[2])
                    if g == 0:
                        den_pairs(pts[0], dens[0], range(NPAIR))
                        rgbgs[0] = den_tail(0, dens[0])
                        xfs[1] = xf_fetch(1)
                    if g == 1:
                        av_tail_full(0, 0)
                    if g == 2:
                        new_den(1)
                        den_pairs(pts[1], dens[1], range(NPAIR))
                        rgbgs[1] = den_tail(1, dens[1])
                    if g == 3:
                        av_tail_full(0, 1)
                    if g == 4:
                        xfs[2] = xf_fetch(2)
                    if g >= 1:
                        den_pairs(pts[2], dens[2], (2 * (g - 1), 2 * (g - 1) + 1))
                den_pairs(pts[2], dens[2], (14, 15))

                # blocks 3..7: steady state (block 3 also carries av(2))
                for n in range(3, NIB):
                    new_block(n)
                    new_den(n)
                    pm = n - 1 if n > 3 else 1
                    for g in range(NG):
                        energy(n, g, pts[n])
                        if g == 0:
                            rgbgs[n - 1] = den_tail(n - 1, dens[n - 1])
                            if n <= NIB - 2:
                                xfs[n] = xf_fetch(n)
                        if g == 1:
                            av_tail_full(pm, 0)
                        if g == 3:
                            av_tail_full(pm, 1)
                        if g == 5 and n == 3:
                            av_tail_full(2, 0)
                        if g == 7 and n == 3:
                            av_tail_full(2, 1)
                        if g == 6 and n == NIB - 1:
                            xfs[7] = xf_fetch(7)
                        # eager denominator for this block (1-group lag)
                        if g >= 1:
                            den_pairs(pts[n], dens[n],
                                      (2 * (g - 1), 2 * (g - 1) + 1))
                        # last block: eager AV so the drain is short
                        if n == NIB - 1:
                            if g >= 3:
                                if g == 3:
                                    new_av(7, 0)
                                av_pairs(pts[7], avs[(7, 0)], 0,
                                         (2 * (g - 3), 2 * (g - 3) + 1))
                            if g >= 5:
                                if g == 5:
                                    new_av(7, 1)
                                av_pairs(pts[7], avs[(7, 1)], 1,
                                         (2 * (g - 5), 2 * (g - 5) + 1))
                    den_pairs(pts[n], dens[n], (14, 15))

                # drain: finish block 7
                av_pairs(pts[7], avs[(7, 0)], 0, range(10, NPAIR))
                av_pairs(pts[7], avs[(7, 1)], 1, range(6, NPAIR))
                rgbgs[7] = den_tail(7, dens[7])
                tail_ch(7, 0, avs[(7, 0)], rgbgs[7], xfs[7][0])
                tail_ch(7, 1, avs[(7, 1)], rgbgs[7], xfs[7][1])
    nc.compile()
    return nc


_NC_CACHE = {}


def kernel(x, y, Wq, bq, Wk, bk, Wv, bv, gamma):
    assert x.shape == (B, C, 64, 64)
    xs = np.ascontiguousarray(x.reshape(B, C, HW)).astype(np.float32)
    ys = np.ascontiguousarray(y.reshape(B, C, HW)).astype(np.float32)

    # x: fp16, chunk-major [p, (g, h, 512)] for one-DMA-per-chunk loads
    x16 = (xs.reshape(B, 2, 128, NIB, IBLK).transpose(0, 2, 3, 1, 4)
           .reshape(B, 128, CH * HW).astype(np.float16))
    # y: fp8, [p, (g, h, s, 512)] with channel c = h*128 + s*64 + p
    y8 = (ys.reshape(B, 2, 2, 64, NIB, IBLK).transpose(0, 3, 4, 1, 2, 5)
          .reshape(B, 64, 4 * HW).astype(F8NP))
    wq16 = (Wq.T.reshape(2, 128, D).transpose(1, 0, 2)
            .reshape(128, CH * D).astype(np.float16))
    wk8 = (Wk.T.reshape(2, 2, 64, D).transpose(2, 0, 1, 3)
           .reshape(64, 4 * D).astype(F8NP))
    wv8 = (Wv.T.reshape(2, 2, 64, C).transpose(2, 0, 1, 3)
           .reshape(64, 4 * C).astype(F8NP))
    bqh = np.ascontiguousarray(bq.astype(np.float32).reshape(D, 1))
    bkh = np.ascontiguousarray(bk.astype(np.float32).reshape(D, 1))
    g = float(np.asarray(gamma).reshape(-1)[0])
    gbvh = np.ascontiguousarray((g * bv.astype(np.float32)).reshape(CH, 128).T)
    gmh = np.full((128, 1), g, dtype=np.float32)

    if "nc" not in _NC_CACHE:
        _NC_CACHE["nc"] = _build()
    nc = _NC_CACHE["nc"]

    in_maps = [
        {
            "x16d": np.ascontiguousarray(x16[b]),
            "y8d": np.ascontiguousarray(y8[b]),
            "xfd": np.ascontiguousarray(xs[b]),
            "wq16d": np.ascontiguousarray(wq16),
            "wk8d": np.ascontiguousarray(wk8),
            "wv8d": np.ascontiguousarray(wv8),
            "bqd": bqh, "bkd": bkh, "gbvd": gbvh, "gmd": gmh,
        }
        for b in range(B)
    ]
    res = run_bass_kernel_spmd(nc, in_maps, list(range(B)))
    outs = np.stack([res.results[b]["out"] for b in range(B)])
    return outs.reshape(B, C, 64, 64).astype(np.float32)


# revision 4
# speedup vs baseline: 1.1995x; 1.1104x over previous
"""Cross-modal attention kernel for Trainium2 -- data-parallel over batch on 8 cores.

Reference computation per sample (C=256, H=W=64, N=H*W=4096, dqk=32):
    q = Wq @ x + bq; k = Wk @ y + bk; v = Wv @ y + bv
    out = gamma * (v @ softmax_j(q^T k)^T) + x

The Activation engine is the roofline: exp() over the [N, N] energy matrix is
131072 columns at 0.8333 ns/col (~133 us including per-instruction access
overhead), so the whole schedule keeps exp() streaming gap-free from ~6 us on:

  - Projections are FUSED into the first two i-blocks: k chunks are projected
    just ahead of block 0's energy matmuls, q chunks 0-1 in block 0 and the
    rest plus all of v during block 1.  No separate phase, no PE idle gap.
  - q/k are built directly in the [16, 2, N] DoubleRow layout: each 256-col
    sub-chunk is four halved-lhsT matmuls into a [16, 2, 256] PSUM tile, the
    bias is added as a K=1 matmul row (bias.T @ ones), and one 3D-AP DVE
    tensor_copy converts f32 -> fp8.  No partition-shuffling DMAs anywhere.
  - Inputs are uploaded pre-quantized (x fp16, y fp8e4m3): ramp DMA is 3 MB
    instead of 12 MB.  The f32 residual x streams in later, when DMA is idle.
  - Every attention matmul runs fp8 DoubleRow (0.5 cyc/row): energy contracts
    q/k [16, 2, *], k/v projections contract y [64, 2, *], AV and the softmax
    denominator pair consecutive j-tiles [128, 2, *].  PE busy ~70 us.
  - PSUM (8 banks): et double-buffer 4 + kq ring 2 + pv ring 2 during blocks
    0-1, then et 4 + den ring 2 + av 2.  den for blocks 0/1 and AV for every
    block run as deferred 16-step DoubleRow bursts one block later (pt lives
    in a 3-deep SBUF ring).  The last block's AV runs eagerly (ch1 borrows
    the freed den-ring bank) and its tail is processed in half-blocks, so the
    post-exp drain is short.
  - Softmax normalization happens on the [C, IBLK] output (reciprocal of the
    ones-matmul denominator), never on the [N, N] matrix.
"""

import sys

if "/opt/trn_rl_repo" not in sys.path:
    sys.path.insert(0, "/opt/trn_rl_repo")

import ml_dtypes
import numpy as np

import concourse.bacc as bacc
import concourse.mybir as mybir
import concourse.tile as tile
from concourse.bass_utils import run_bass_kernel_spmd

F32 = mybir.dt.float32
F16 = mybir.dt.float16
BF16 = mybir.dt.bfloat16
F8 = mybir.dt.float8e4
F8NP = ml_dtypes.float8_e4m3
BF16NP = ml_dtypes.bfloat16

B, C, HW, D = 8, 256, 4096, 32
CH = C // 128
IBLK = 512
NIB = HW // IBLK          # 8 i-blocks
NJT = HW // 128           # 32 j-tiles
NPAIR = NJT // 2          # 16 DoubleRow pairs
NG = NJT // 4             # 8 energy groups per block (4 j-tiles each)
NSC = HW // 256           # 16 projection sub-chunks
EXPF = mybir.ActivationFunctionType.Exp
MULT = mybir.AluOpType.mult
ADD = mybir.AluOpType.add
DROW = mybir.MatmulPerfMode.DoubleRow


def _build():
    nc = bacc.Bacc("TRN2", target_bir_lowering=False, debug=False, num_devices=8)

    x16d = nc.dram_tensor("x16d", [128, CH * HW], F16, kind="ExternalInput")
    y8d = nc.dram_tensor("y8d", [64, 4 * HW], F8, kind="ExternalInput")
    xfd = nc.dram_tensor("xfd", [C, HW], F32, kind="ExternalInput")
    wq16d = nc.dram_tensor("wq16d", [128, CH * D], F16, kind="ExternalInput")
    wk8d = nc.dram_tensor("wk8d", [64, 4 * D], F8, kind="ExternalInput")
    wv8d = nc.dram_tensor("wv8d", [64, 4 * C], F8, kind="ExternalInput")
    bqrd = nc.dram_tensor("bqrd", [1, D], BF16, kind="ExternalInput")
    bkrd = nc.dram_tensor("bkrd", [1, D], BF16, kind="ExternalInput")
    gbvd = nc.dram_tensor("gbvd", [128, CH], F32, kind="ExternalInput")
    gmd = nc.dram_tensor("gmd", [128, 1], F32, kind="ExternalInput")
    out = nc.dram_tensor("out", [C, HW], F32, kind="ExternalOutput")

    tc = tile.TileContext(nc)
    with tc:
        with (
            tc.tile_pool(name="cst", bufs=1) as cst,
            tc.tile_pool(name="ptp", bufs=3) as ptp,
            tc.tile_pool(name="wrk", bufs=2) as wrk,
            tc.tile_pool(name="psE", bufs=1, space="PSUM") as psE,
        ):
            wq_sb = cst.tile([128, CH * D], F16)
            wk_sb = cst.tile([64, 4 * D], F8)
            wv_sb = cst.tile([64, 4 * C], F8)
            bq_row = cst.tile([1, D], BF16)
            bk_row = cst.tile([1, D], BF16)
            ones_row = cst.tile([1, 256], BF16)
            gbv_sb = cst.tile([128, CH], F32)
            gm_sb = cst.tile([128, 1], F32)
            ones_sb = cst.tile([128, 2 * 128], F8)
            x_sb = cst.tile([128, CH * HW], F16)
            y_sb = cst.tile([64, 4 * HW], F8)
            q4f = cst.tile([16, 2 * HW], F8)
            k4f = cst.tile([16, 2 * HW], F8)
            vt = cst.tile([128, NJT * C], F8)

            nc.vector.memset(ones_sb[:], 1.0)
            nc.vector.memset(ones_row[:], 1.0)
            nc.gpsimd.dma_start(bq_row[:], bqrd[:])
            nc.gpsimd.dma_start(bk_row[:], bkrd[:])
            nc.gpsimd.dma_start(gbv_sb[:], gbvd[:])
            nc.gpsimd.dma_start(gm_sb[:], gmd[:])

            # weights + inputs on the SP queue; k-path (wk, y) leads
            nc.sync.dma_start(wk_sb[:], wk8d[:])
            y4 = y_sb[:].rearrange("P (b N) -> P b N", b=4)
            x2 = x_sb[:].rearrange("P (h N) -> P h N", h=2)
            nc.sync.dma_start(
                y4[:, :, 0:IBLK],
                y8d[:, 0:4 * IBLK].rearrange("P (b N) -> P b N", b=4))
            nc.sync.dma_start(wq_sb[:], wq16d[:])
            nc.sync.dma_start(
                x2[:, :, 0:IBLK],
                x16d[:, 0:2 * IBLK].rearrange("P (h N) -> P h N", h=2))
            nc.sync.dma_start(wv_sb[:], wv8d[:])
            for g in range(1, NIB):
                nc.sync.dma_start(
                    y4[:, :, g * IBLK:(g + 1) * IBLK],
                    y8d[:, g * 4 * IBLK:(g + 1) * 4 * IBLK].rearrange(
                        "P (b N) -> P b N", b=4))
                nc.sync.dma_start(
                    x2[:, :, g * IBLK:(g + 1) * IBLK],
                    x16d[:, g * 2 * IBLK:(g + 1) * 2 * IBLK].rearrange(
                        "P (h N) -> P h N", h=2))

            q4r = q4f[:].rearrange("P (s N) -> P s N", s=2)
            k4r = k4f[:].rearrange("P (s N) -> P s N", s=2)
            wk4 = wk_sb[:].rearrange("P (b d) -> P b d", b=4)
            wv4 = wv_sb[:].rearrange("P (b c) -> P b c", b=4)
            ones_pair = ones_sb[:].rearrange("P (s c) -> P s c", s=2)

            def energy(n, g, pt):
                # 4 j-tiles of E^T[j, i-block n] as fp8 DoubleRow matmuls,
                # exp() into pt in two [128, 1024] activations
                c0, c1 = n * IBLK, (n + 1) * IBLK
                for h in range(2):
                    et = psE.tile([128, 2 * IBLK], F32,
                                  name=f"et{h}_{n}_{g}", tag="et", bufs=2)
                    for t in range(2):
                        jt = 4 * g + 2 * h + t
                        nc.tensor.matmul(
                            et[:, t * IBLK:(t + 1) * IBLK],
                            k4r[:, :, jt * 128:(jt + 1) * 128],
                            q4r[:, :, c0:c1],
                            start=True, stop=True,
                            perf_mode=DROW,
                        )
                    nc.scalar.activation(
                        pt[:, (4 * g + 2 * h) * IBLK:(4 * g + 2 * h + 2) * IBLK],
                        et[:], EXPF,
                    )

            def ptp_ap(pt, p):
                return pt[:, 2 * p * IBLK:(2 * p + 2) * IBLK].rearrange(
                    "P (s N) -> P s N", s=2)

            def den_pairs(pt, den, pairs):
                for p in pairs:
                    nc.tensor.matmul(
                        den[:], ones_pair, ptp_ap(pt, p),
                        start=(p == 0), stop=(p == NPAIR - 1),
                        perf_mode=DROW, skip_group_check=True,
                    )

            def av_pairs(pt, av, ch, pairs):
                for p in pairs:
                    nc.tensor.matmul(
                        av[:],
                        vt[:, 2 * p * C:(2 * p + 2) * C].rearrange(
                            "P (s c) -> P s c", s=2)[:, :, ch * 128:(ch + 1) * 128],
                        ptp_ap(pt, p),
                        start=(p == 0), stop=(p == NPAIR - 1),
                        perf_mode=DROW, skip_group_check=True,
                    )

            def den_tail(n, den):
                # rgbg = gamma / den, shared by both C-chunks of block n
                rgb = wrk.tile([128, IBLK], F32, name=f"rgb_{n}", tag="rgb")
                nc.vector.reciprocal(rgb[:], den[:])
                rgbg = wrk.tile([128, IBLK], F32, name=f"rgbg_{n}", tag="rgbg",
                                bufs=3)
                nc.vector.tensor_scalar(rgbg[:], rgb[:], gm_sb[:, 0:1], None, MULT)
                return rgbg

            def xf_fetch(n):
                xs = []
                for ch in range(CH):
                    xf_t = wrk.tile([128, IBLK], F32, name=f"xf_{n}_{ch}",
                                    tag="xf", bufs=6)
                    nc.gpsimd.dma_start(
                        xf_t[:],
                        xfd[ch * 128:(ch + 1) * 128, n * IBLK:(n + 1) * IBLK],
                    )
                    xs.append(xf_t)
                return xs

            def tail_ch(n, ch, av, rgbg, xf_t):
                tmp = wrk.tile([128, IBLK], F32, name=f"tmp_{n}_{ch}", tag="tmp")
                nc.vector.tensor_tensor(tmp[:], av[:], rgbg[:], MULT)
                ot = wrk.tile([128, IBLK], F32, name=f"ot_{n}_{ch}", tag="ot",
                              bufs=3)
                nc.vector.scalar_tensor_tensor(
                    ot[:], tmp[:], gbv_sb[:, ch:ch + 1], xf_t[:], ADD, ADD)
                nc.sync.dma_start(
                    out[ch * 128:(ch + 1) * 128, n * IBLK:(n + 1) * IBLK], ot[:])

            pts = {}
            dens = {}
            avs = {}
            rgbgs = {}
            xfs = {}

            def new_block(n):
                pts[n] = ptp.tile([128, NJT * IBLK], F8, name=f"pt_{n}", tag="pt")

            # ------------- blocks 0-1: projections + energy ---------------
            with tc.tile_pool(name="psP", bufs=1, space="PSUM") as psP:
                def k_proj(sc):
                    c0, c1 = sc * 256, (sc + 1) * 256
                    ps = psP.tile([16, 2 * 256], F32, name=f"kps_{sc}",
                                  tag="kq_ps", bufs=2)
                    ps3 = ps.rearrange("P (s N) -> P s N", s=2)
                    for s in range(2):
                        for h in range(CH):
                            nc.tensor.matmul(
                                ps3[:, s, :],
                                wk4[:, 2 * h:2 * h + 2, 16 * s:16 * (s + 1)],
                                y4[:, 2 * h:2 * h + 2, c0:c1],
                                start=(h == 0), stop=False, perf_mode=DROW,
                                skip_group_check=True)
                        nc.tensor.matmul(
                            ps3[:, s, :], bk_row[0:1, 16 * s:16 * (s + 1)],
                            ones_row[:], start=False, stop=True,
                            skip_group_check=True)
                    nc.vector.tensor_copy(k4r[:, :, c0:c1], ps3[:])

                def q_proj(sc):
                    c0, c1 = sc * 256, (sc + 1) * 256
                    scb = (sc * 256) // IBLK  # x chunk containing these cols
                    ps = psP.tile([16, 2 * 256], F32, name=f"qps_{sc}",
                                  tag="kq_ps", bufs=2)
                    ps3 = ps.rearrange("P (s N) -> P s N", s=2)
                    for s in range(2):
                        for h in range(CH):
                            nc.tensor.matmul(
                                ps3[:, s, :],
                                wq_sb[:, h * D + 16 * s:h * D + 16 * (s + 1)],
                                x2[:, h, c0:c1],
                                start=(h == 0), stop=False,
                                skip_group_check=True)
                        nc.tensor.matmul(
                            ps3[:, s, :], bq_row[0:1, 16 * s:16 * (s + 1)],
                            ones_row[:], start=False, stop=True,
                            skip_group_check=True)
                    nc.vector.tensor_copy(q4r[:, :, c0:c1], ps3[:])

                def v_proj(vp):
                    # one pv tile = 2 j-tiles
                    ps = psP.tile([128, IBLK], F32, name=f"vps_{vp}",
                                  tag="pv_ps", bufs=2)
                    for t in range(2):
                        jt = 2 * vp + t
                        for h in range(CH):
                            nc.tensor.matmul(
                                ps[:, t * 256:(t + 1) * 256],
                                y4[:, 2 * h:2 * h + 2, jt * 128:(jt + 1) * 128],
                                wv4[:, 2 * h:2 * h + 2, :],
                                start=(h == 0), stop=(h == CH - 1),
                                perf_mode=DROW, skip_group_check=True)
                    nc.vector.tensor_copy(
                        vt[:, 2 * vp * C:(2 * vp + 2) * C], ps[:])

                # block 0: all k sub-chunks + q chunks 0-1, one ahead of energy
                new_block(0)
                for g in range(NG):
                    if g == 0:
                        k_proj(0)
                        k_proj(1)
                        q_proj(0)
                        q_proj(1)
                        k_proj(2)
                        k_proj(3)
                        q_proj(2)
                        q_proj(3)
                    elif g <= NG - 2:
                        k_proj(2 * g + 2)
                        k_proj(2 * g + 3)
                    energy(0, g, pts[0])

                # block 1: energy + q chunks 2-7 + all v
                new_block(1)
                for g in range(NG):
                    energy(1, g, pts[1])
                    if g < 6:
                        q_proj(2 * g + 4)
                        q_proj(2 * g + 5)
                    v_proj(2 * g)
                    v_proj(2 * g + 1)
                xfs[0] = xf_fetch(0)

            # ------------- blocks 2..7 + deferred den/av ------------------
            with tc.tile_pool(name="psAV", bufs=1, space="PSUM") as psAV:
                def new_den(n):
                    dens[n] = psAV.tile([128, IBLK], F32, name=f"den_{n}",
                                        tag="den", bufs=2)

                def new_av(n, ch):
                    avs[(n, ch)] = psAV.tile([128, IBLK], F32,
                                             name=f"av{ch}_{n}",
                                             tag=f"av{ch}", bufs=1)

                def av_tail_full(m, ch):
                    new_av(m, ch)
                    av_pairs(pts[m], avs[(m, ch)], ch, range(NPAIR))
                    tail_ch(m, ch, avs[(m, ch)], rgbgs[m], xfs[m][ch])

                # block 2: den(0)/den(1) bursts, av(0), eager den(2)
                new_block(2)
                new_den(0)   # den ring order: 0 -> bufA, 2 -> bufB, 1 -> bufA
                new_den(2)
                for g in range(NG):
                    energy(2, g, pts[2])
                    if g == 0:
                        den_pairs(pts[0], dens[0], range(NPAIR))
                        rgbgs[0] = den_tail(0, dens[0])
                        xfs[1] = xf_fetch(1)
                    if g == 2:
                        av_tail_full(0, 0)
                    if g == 3:
                        new_den(1)
                        den_pairs(pts[1], dens[1], range(NPAIR))
                        rgbgs[1] = den_tail(1, dens[1])
                    if g == 4:
                        av_tail_full(0, 1)
                        xfs[2] = xf_fetch(2)
                    if g >= 1:
                        den_pairs(pts[2], dens[2], (2 * (g - 1), 2 * (g - 1) + 1))
                den_pairs(pts[2], dens[2], (14, 15))

                # blocks 3..7: steady state (block 3 also carries av(2))
                for n in range(3, NIB):
                    new_block(n)
                    new_den(n)
                    pm = n - 1 if n > 3 else 1
                    last = n == NIB - 1
                    for g in range(NG):
                        energy(n, g, pts[n])
                        if g == 0:
                            rgbgs[n - 1] = den_tail(n - 1, dens[n - 1])
                            if n <= NIB - 2:
                                xfs[n] = xf_fetch(n)
                        if g == 1:
                            av_tail_full(pm, 0)
                        if g == 3:
                            av_tail_full(pm, 1)
                        if g == 5 and n == 3:
                            av_tail_full(2, 0)
                        if g == 7 and n == 3:
                            av_tail_full(2, 1)
                        if g == 6 and last:
                            xfs[7] = xf_fetch(7)
                        # eager denominator for this block (1-group lag)
                        if g >= 1:
                            den_pairs(pts[n], dens[n],
                                      (2 * (g - 1), 2 * (g - 1) + 1))
                        # last block: eager AV so the drain is short.
                        # ch1 borrows the freed den-ring bank (after recip(6)).
                        if last:
                            if g >= 2:
                                if g == 2:
                                    avs[(7, 1)] = psAV.tile(
                                        [128, IBLK], F32, name="av1_7",
                                        tag="den", bufs=2)
                                av_pairs(pts[7], avs[(7, 1)], 1,
                                         (2 * (g - 2), 2 * (g - 2) + 1))
                            if g >= 3:
                                if g == 3:
                                    new_av(7, 0)
                                av_pairs(pts[7], avs[(7, 0)], 0,
                                         (2 * (g - 3), 2 * (g - 3) + 1))
                    den_pairs(pts[n], dens[n], (14, 15))

                # drain: finish block 7, tail in half-blocks to overlap DMA
                av_pairs(pts[7], avs[(7, 1)], 1, range(12, NPAIR))
                av_pairs(pts[7], avs[(7, 0)], 0, range(10, NPAIR))
                den7 = dens[7]
                for hf in range(2):
                    f0, f1 = hf * 256, (hf + 1) * 256
                    rgb = wrk.tile([128, 256], F32, name=f"rgb7_{hf}", tag="rgb7")
                    nc.vector.reciprocal(rgb[:], den7[:, f0:f1])
                    rgbg = wrk.tile([128, 256], F32, name=f"rgbg7_{hf}",
                                    tag="rgbg7")
                    nc.vector.tensor_scalar(rgbg[:], rgb[:], gm_sb[:, 0:1],
                                            None, MULT)
                    for ch in range(CH):
                        tmp = wrk.tile([128, 256], F32, name=f"tmp7_{hf}_{ch}",
                                       tag="tmp7")
                        nc.vector.tensor_tensor(
                            tmp[:], avs[(7, ch)][:, f0:f1], rgbg[:], MULT)
                        ot = wrk.tile([128, 256], F32, name=f"ot7_{hf}_{ch}",
                                      tag="ot7")
                        nc.vector.scalar_tensor_tensor(
                            ot[:], tmp[:], gbv_sb[:, ch:ch + 1],
                            xfs[7][ch][:, f0:f1], ADD, ADD)
                        nc.sync.dma_start(
                            out[ch * 128:(ch + 1) * 128,
                                7 * IBLK + f0:7 * IBLK + f1], ot[:])
    nc.compile()
    return nc


_NC_CACHE = {}


def kernel(x, y, Wq, bq, Wk, bk, Wv, bv, gamma):
    assert x.shape == (B, C, 64, 64)
    xs = np.ascontiguousarray(x.reshape(B, C, HW)).astype(np.float32)
    ys = np.ascontiguousarray(y.reshape(B, C, HW)).astype(np.float32)

    # x: fp16, chunk-major [p, (g, h, 512)] for one-DMA-per-chunk loads
    x16 = (xs.reshape(B, 2, 128, NIB, IBLK).transpose(0, 2, 3, 1, 4)
           .reshape(B, 128, CH * HW).astype(np.float16))
    # y: fp8, [p, (g, h, s, 512)] with channel c = h*128 + s*64 + p
    y8 = (ys.reshape(B, 2, 2, 64, NIB, IBLK).transpose(0, 3, 4, 1, 2, 5)
          .reshape(B, 64, 4 * HW).astype(F8NP))
    wq16 = (Wq.T.reshape(2, 128, D).transpose(1, 0, 2)
            .reshape(128, CH * D).astype(np.float16))
    wk8 = (Wk.T.reshape(2, 2, 64, D).transpose(2, 0, 1, 3)
           .reshape(64, 4 * D).astype(F8NP))
    wv8 = (Wv.T.reshape(2, 2, 64, C).transpose(2, 0, 1, 3)
           .reshape(64, 4 * C).astype(F8NP))
    bqr = np.ascontiguousarray(bq.reshape(1, D)).astype(BF16NP)
    bkr = np.ascontiguousarray(bk.reshape(1, D)).astype(BF16NP)
    g = float(np.asarray(gamma).reshape(-1)[0])
    gbvh = np.ascontiguousarray((g * bv.astype(np.float32)).reshape(CH, 128).T)
    gmh = np.full((128, 1), g, dtype=np.float32)

    if "nc" not in _NC_CACHE:
        _NC_CACHE["nc"] = _build()
    nc = _NC_CACHE["nc"]

    in_maps = [
        {
            "x16d": np.ascontiguousarray(x16[b]),
            "y8d": np.ascontiguousarray(y8[b]),
            "xfd": np.ascontiguousarray(xs[b]),
            "wq16d": np.ascontiguousarray(wq16),
            "wk8d": np.ascontiguousarray(wk8),
            "wv8d": np.ascontiguousarray(wv8),
            "bqrd": bqr, "bkrd": bkr, "gbvd": gbvh, "gmd": gmh,
        }
        for b in range(B)
    ]
    res = run_bass_kernel_spmd(nc, in_maps, list(range(B)))
    outs = np.stack([res.results[b]["out"] for b in range(B)])
    return outs.reshape(B, C, 64, 64).astype(np.float32)


# revision 8
# speedup vs baseline: 1.2190x; 1.0162x over previous
"""Cross-modal attention kernel for Trainium2 -- data-parallel over batch on 8 cores.

Reference computation per sample (C=256, H=W=64, N=H*W=4096, dqk=32):
    q = Wq @ x + bq; k = Wk @ y + bk; v = Wv @ y + bv
    out = gamma * (v @ softmax_j(q^T k)^T) + x

The Activation engine is the roofline: exp() over the [N, N] energy matrix is
131072 columns at 0.8333 ns/col (~133 us including per-instruction access
overhead), so the whole schedule keeps exp() streaming gap-free from ~6 us on:

  - Projections are FUSED into the first two i-blocks: k chunks are projected
    just ahead of block 0's energy matmuls, q chunks 0-1 in block 0 and the
    rest plus all of v during block 1.  No separate phase, no PE idle gap.
  - q/k are built directly in the [16, 2, N] DoubleRow layout: each 256-col
    sub-chunk is four halved-lhsT matmuls into a [16, 2, 256] PSUM tile, the
    bias is added as a K=1 matmul row (bias.T @ ones), and one 3D-AP DVE
    tensor_copy converts f32 -> fp8.  No partition-shuffling DMAs anywhere.
  - Inputs are uploaded pre-quantized (x fp16, y fp8e4m3): ramp DMA is 3 MB
    instead of 12 MB.  The f32 residual x streams in later, when DMA is idle.
  - Every attention matmul runs fp8 DoubleRow (0.5 cyc/row): energy contracts
    q/k [16, 2, *], k/v projections contract y [64, 2, *], AV and the softmax
    denominator pair consecutive j-tiles [128, 2, *].  PE busy ~70 us.
  - PSUM (8 banks): et double-buffer 4 + kq ring 2 + pv ring 2 during blocks
    0-1, then et 4 + den ring 2 + av 2.  den for blocks 0/1 and AV for every
    block run as deferred 16-step DoubleRow bursts one block later (pt lives
    in a 3-deep SBUF ring).  The last block's AV runs eagerly (ch1 borrows
    the freed den-ring bank) and its tail is processed in half-blocks, so the
    post-exp drain is short.
  - Softmax normalization happens on the [C, IBLK] output (reciprocal of the
    ones-matmul denominator), never on the [N, N] matrix.
"""

import sys

if "/opt/trn_rl_repo" not in sys.path:
    sys.path.insert(0, "/opt/trn_rl_repo")

import ml_dtypes
import numpy as np

import concourse.bacc as bacc
import concourse.mybir as mybir
import concourse.tile as tile
from concourse.bass_utils import run_bass_kernel_spmd

F32 = mybir.dt.float32
F16 = mybir.dt.float16
BF16 = mybir.dt.bfloat16
F8 = mybir.dt.float8e4
F8NP = ml_dtypes.float8_e4m3
BF16NP = ml_dtypes.bfloat16

B, C, HW, D = 8, 256, 4096, 32
CH = C // 128
IBLK = 512
NIB = HW // IBLK          # 8 i-blocks
NJT = HW // 128           # 32 j-tiles
NPAIR = NJT // 2          # 16 DoubleRow pairs
NG = NJT // 4             # 8 energy groups per block (4 j-tiles each)
NSC = HW // 256           # 16 projection sub-chunks
EXPF = mybir.ActivationFunctionType.Exp
MULT = mybir.AluOpType.mult
ADD = mybir.AluOpType.add
DROW = mybir.MatmulPerfMode.DoubleRow


def _build():
    nc = bacc.Bacc("TRN2", target_bir_lowering=False, debug=False, num_devices=8)

    x16d = nc.dram_tensor("x16d", [128, CH * HW], F16, kind="ExternalInput")
    y8d = nc.dram_tensor("y8d", [64, 4 * HW], F8, kind="ExternalInput")
    xfd = nc.dram_tensor("xfd", [C, HW], F32, kind="ExternalInput")
    wq16d = nc.dram_tensor("wq16d", [128, CH * D], F16, kind="ExternalInput")
    wk8d = nc.dram_tensor("wk8d", [64, 4 * D], F8, kind="ExternalInput")
    wv8d = nc.dram_tensor("wv8d", [64, 4 * C], F8, kind="ExternalInput")
    bqrd = nc.dram_tensor("bqrd", [1, D], BF16, kind="ExternalInput")
    bkrd = nc.dram_tensor("bkrd", [1, D], BF16, kind="ExternalInput")
    gbvd = nc.dram_tensor("gbvd", [128, CH], F32, kind="ExternalInput")
    out = nc.dram_tensor("out", [C, HW], F32, kind="ExternalOutput")

    tc = tile.TileContext(nc)
    with tc:
        with (
            tc.tile_pool(name="cst", bufs=1) as cst,
            tc.tile_pool(name="ptp", bufs=3) as ptp,
            tc.tile_pool(name="wrk", bufs=2) as wrk,
            tc.tile_pool(name="psE", bufs=1, space="PSUM") as psE,
        ):
            wq_sb = cst.tile([128, CH * D], F16)
            wk_sb = cst.tile([64, 4 * D], F8)
            wv_sb = cst.tile([64, 4 * C], F8)
            bq_row = cst.tile([1, D], BF16)
            bk_row = cst.tile([1, D], BF16)
            ones_row = cst.tile([1, 256], BF16)
            gbv_sb = cst.tile([128, CH], F32)
            ones_sb = cst.tile([128, 2 * 128], F8)
            x_sb = cst.tile([128, CH * HW], F16)
            y_sb = cst.tile([64, 4 * HW], F8)
            q4f = cst.tile([16, 2 * HW], F8)
            k4f = cst.tile([16, 2 * HW], F8)
            vt = cst.tile([128, NJT * C], F8)

            nc.vector.memset(ones_sb[:], 1.0)
            nc.vector.memset(ones_row[:], 1.0)
            nc.gpsimd.dma_start(bq_row[:], bqrd[:])
            nc.gpsimd.dma_start(bk_row[:], bkrd[:])
            nc.gpsimd.dma_start(gbv_sb[:], gbvd[:])

            # weights + inputs on the SP queue; k-path (wk, y) leads
            nc.sync.dma_start(wk_sb[:], wk8d[:])
            y4 = y_sb[:].rearrange("P (b N) -> P b N", b=4)
            x2 = x_sb[:].rearrange("P (h N) -> P h N", h=2)
            nc.sync.dma_start(
                y4[:, :, 0:IBLK],
                y8d[:, 0:4 * IBLK].rearrange("P (b N) -> P b N", b=4))
            nc.sync.dma_start(wq_sb[:], wq16d[:])
            nc.sync.dma_start(
                x2[:, :, 0:IBLK],
                x16d[:, 0:2 * IBLK].rearrange("P (h N) -> P h N", h=2))
            nc.sync.dma_start(wv_sb[:], wv8d[:])
            for g in range(1, NIB):
                nc.sync.dma_start(
                    y4[:, :, g * IBLK:(g + 1) * IBLK],
                    y8d[:, g * 4 * IBLK:(g + 1) * 4 * IBLK].rearrange(
                        "P (b N) -> P b N", b=4))
                nc.sync.dma_start(
                    x2[:, :, g * IBLK:(g + 1) * IBLK],
                    x16d[:, g * 2 * IBLK:(g + 1) * 2 * IBLK].rearrange(
                        "P (h N) -> P h N", h=2))

            q4r = q4f[:].rearrange("P (s N) -> P s N", s=2)
            k4r = k4f[:].rearrange("P (s N) -> P s N", s=2)
            wk4 = wk_sb[:].rearrange("P (b d) -> P b d", b=4)
            wv4 = wv_sb[:].rearrange("P (b c) -> P b c", b=4)
            ones_pair = ones_sb[:].rearrange("P (s c) -> P s c", s=2)

            def energy(n, g, pt):
                # 4 j-tiles of E^T[j, i-block n] as fp8 DoubleRow matmuls,
                # exp() into pt in two [128, 1024] activations
                c0, c1 = n * IBLK, (n + 1) * IBLK
                for h in range(2):
                    et = psE.tile([128, 2 * IBLK], F32,
                                  name=f"et{h}_{n}_{g}", tag="et", bufs=2)
                    for t in range(2):
                        jt = 4 * g + 2 * h + t
                        nc.tensor.matmul(
                            et[:, t * IBLK:(t + 1) * IBLK],
                            k4r[:, :, jt * 128:(jt + 1) * 128],
                            q4r[:, :, c0:c1],
                            start=True, stop=True,
                            perf_mode=DROW,
                        )
                    nc.scalar.activation(
                        pt[:, (4 * g + 2 * h) * IBLK:(4 * g + 2 * h + 2) * IBLK],
                        et[:], EXPF,
                    )

            def ptp_ap(pt, p):
                return pt[:, 2 * p * IBLK:(2 * p + 2) * IBLK].rearrange(
                    "P (s N) -> P s N", s=2)

            def den_pairs(pt, den, pairs):
                for p in pairs:
                    nc.tensor.matmul(
                        den[:], ones_pair, ptp_ap(pt, p),
                        start=(p == 0), stop=(p == NPAIR - 1),
                        perf_mode=DROW, skip_group_check=True,
                    )

            def av_pairs(pt, av, ch, pairs):
                for p in pairs:
                    nc.tensor.matmul(
                        av[:],
                        vt[:, 2 * p * C:(2 * p + 2) * C].rearrange(
                            "P (s c) -> P s c", s=2)[:, :, ch * 128:(ch + 1) * 128],
                        ptp_ap(pt, p),
                        start=(p == 0), stop=(p == NPAIR - 1),
                        perf_mode=DROW, skip_group_check=True,
                    )

            def den_tail(n, den):
                # rgb = 1 / den, shared by both C-chunks of block n
                # (gamma is folded into Wv/bv on the host)
                rgb = wrk.tile([128, IBLK], F32, name=f"rgb_{n}", tag="rgb",
                               bufs=3)
                nc.vector.reciprocal(rgb[:], den[:])
                return rgb

            def xf_fetch(n):
                xs = []
                for ch in range(CH):
                    xf_t = wrk.tile([128, IBLK], F32, name=f"xf_{n}_{ch}",
                                    tag="xf", bufs=6)
                    nc.gpsimd.dma_start(
                        xf_t[:],
                        xfd[ch * 128:(ch + 1) * 128, n * IBLK:(n + 1) * IBLK],
                    )
                    xs.append(xf_t)
                return xs

            def tail_ch(n, ch, av, rgbg, xf_t):
                tmp = wrk.tile([128, IBLK], F32, name=f"tmp_{n}_{ch}", tag="tmp")
                nc.vector.tensor_tensor(tmp[:], av[:], rgbg[:], MULT)
                ot = wrk.tile([128, IBLK], F32, name=f"ot_{n}_{ch}", tag="ot",
                              bufs=3)
                nc.vector.scalar_tensor_tensor(
                    ot[:], tmp[:], gbv_sb[:, ch:ch + 1], xf_t[:], ADD, ADD)
                nc.sync.dma_start(
                    out[ch * 128:(ch + 1) * 128, n * IBLK:(n + 1) * IBLK], ot[:])

            pts = {}
            dens = {}
            avs = {}
            rgbgs = {}
            xfs = {}

            def new_block(n):
                pts[n] = ptp.tile([128, NJT * IBLK], F8, name=f"pt_{n}", tag="pt")

            # ------------- blocks 0-1: projections + energy ---------------
            with tc.tile_pool(name="psP", bufs=1, space="PSUM") as psP:
                def k_proj(sc):
                    c0, c1 = sc * 256, (sc + 1) * 256
                    ps = psP.tile([16, 2 * 256], F32, name=f"kps_{sc}",
                                  tag="kq_ps", bufs=2)
                    ps3 = ps.rearrange("P (s N) -> P s N", s=2)
                    for s in range(2):
                        for h in range(CH):
                            nc.tensor.matmul(
                                ps3[:, s, :],
                                wk4[:, 2 * h:2 * h + 2, 16 * s:16 * (s + 1)],
                                y4[:, 2 * h:2 * h + 2, c0:c1],
                                start=(h == 0), stop=False, perf_mode=DROW,
                                skip_group_check=True)
                        nc.tensor.matmul(
                            ps3[:, s, :], bk_row[0:1, 16 * s:16 * (s + 1)],
                            ones_row[:], start=False, stop=True,
                            skip_group_check=True)
                    nc.vector.tensor_copy(k4r[:, :, c0:c1], ps3[:])

                def q_proj(sc):
                    c0, c1 = sc * 256, (sc + 1) * 256
                    scb = (sc * 256) // IBLK  # x chunk containing these cols
                    ps = psP.tile([16, 2 * 256], F32, name=f"qps_{sc}",
                                  tag="kq_ps", bufs=2)
                    ps3 = ps.rearrange("P (s N) -> P s N", s=2)
                    for s in range(2):
                        for h in range(CH):
                            nc.tensor.matmul(
                                ps3[:, s, :],
                                wq_sb[:, h * D + 16 * s:h * D + 16 * (s + 1)],
                                x2[:, h, c0:c1],
                                start=(h == 0), stop=False,
                                skip_group_check=True)
                        nc.tensor.matmul(
                            ps3[:, s, :], bq_row[0:1, 16 * s:16 * (s + 1)],
                            ones_row[:], start=False, stop=True,
                            skip_group_check=True)
                    nc.vector.tensor_copy(q4r[:, :, c0:c1], ps3[:])

                def v_proj(vp):
                    # one pv tile = 2 j-tiles
                    ps = psP.tile([128, IBLK], F32, name=f"vps_{vp}",
                                  tag="pv_ps", bufs=2)
                    for t in range(2):
                        jt = 2 * vp + t
                        for h in range(CH):
                            nc.tensor.matmul(
                                ps[:, t * 256:(t + 1) * 256],
                                y4[:, 2 * h:2 * h + 2, jt * 128:(jt + 1) * 128],
                                wv4[:, 2 * h:2 * h + 2, :],
                                start=(h == 0), stop=(h == CH - 1),
                                perf_mode=DROW, skip_group_check=True)
                    nc.vector.tensor_copy(
                        vt[:, 2 * vp * C:(2 * vp + 2) * C], ps[:])

                # block 0: all k sub-chunks + q chunks 0-1, one ahead of energy
                new_block(0)
                for g in range(NG):
                    if g == 0:
                        k_proj(0)
                        k_proj(1)
                        q_proj(0)
                        q_proj(1)
                        k_proj(2)
                        k_proj(3)
                        q_proj(2)
                        q_proj(3)
                    elif g <= NG - 2:
                        k_proj(2 * g + 2)
                        k_proj(2 * g + 3)
                    energy(0, g, pts[0])

                # block 1: energy + q chunks 2-7 + all v
                new_block(1)
                for g in range(NG):
                    energy(1, g, pts[1])
                    if g < 6:
                        q_proj(2 * g + 4)
                        q_proj(2 * g + 5)
                    v_proj(2 * g)
                    v_proj(2 * g + 1)
                xfs[0] = xf_fetch(0)

            # ------------- blocks 2..7 + deferred den/av ------------------
            with tc.tile_pool(name="psAV", bufs=1, space="PSUM") as psAV:
                def new_den(n):
                    dens[n] = psAV.tile([128, IBLK], F32, name=f"den_{n}",
                                        tag="den", bufs=2)

                def new_av(n, ch):
                    avs[(n, ch)] = psAV.tile([128, IBLK], F32,
                                             name=f"av{ch}_{n}",
                                             tag=f"av{ch}", bufs=1)

                def av_tail_full(m, ch):
                    new_av(m, ch)
                    av_pairs(pts[m], avs[(m, ch)], ch, range(NPAIR))
                    tail_ch(m, ch, avs[(m, ch)], rgbgs[m], xfs[m][ch])

                # block 2: den(0)/den(1) bursts, av(0), eager den(2)
                new_block(2)
                new_den(0)   # den ring order: 0 -> bufA, 2 -> bufB, 1 -> bufA
                new_den(2)
                for g in range(NG):
                    energy(2, g, pts[2])
                    if g == 0:
                        den_pairs(pts[0], dens[0], range(NPAIR))
                        rgbgs[0] = den_tail(0, dens[0])
                        xfs[1] = xf_fetch(1)
                    if g == 2:
                        av_tail_full(0, 0)
                    if g == 3:
                        new_den(1)
                        den_pairs(pts[1], dens[1], range(NPAIR))
                        rgbgs[1] = den_tail(1, dens[1])
                    if g == 4:
                        av_tail_full(0, 1)
                        xfs[2] = xf_fetch(2)
                    if g >= 1:
                        den_pairs(pts[2], dens[2], (2 * (g - 1), 2 * (g - 1) + 1))
                den_pairs(pts[2], dens[2], (14, 15))

                # blocks 3..7: steady state (block 3 also carries av(2))
                for n in range(3, NIB):
                    new_block(n)
                    new_den(n)
                    pm = n - 1 if n > 3 else 1
                    last = n == NIB - 1
                    for g in range(NG):
                        energy(n, g, pts[n])
                        if g == 0:
                            rgbgs[n - 1] = den_tail(n - 1, dens[n - 1])
                            if n <= NIB - 2:
                                xfs[n] = xf_fetch(n)
                        if g == 1:
                            av_tail_full(pm, 0)
                        if g == 3:
                            av_tail_full(pm, 1)
                        if g == 5 and n == 3:
                            av_tail_full(2, 0)
                        if g == 7 and n == 3:
                            av_tail_full(2, 1)
                        if g == 6 and last:
                            xfs[7] = xf_fetch(7)
                        # eager denominator for this block (1-group lag)
                        if g >= 1:
                            den_pairs(pts[n], dens[n],
                                      (2 * (g - 1), 2 * (g - 1) + 1))
                        # last block: eager AV so the drain is short.
                        # ch1 borrows the freed den-ring bank (after recip(6)).
                        if last:
                            if g >= 2:
                                if g == 2:
                                    avs[(7, 1)] = psAV.tile(
                                        [128, IBLK], F32, name="av1_7",
                                        tag="den", bufs=2)
                                av_pairs(pts[7], avs[(7, 1)], 1,
                                         (2 * (g - 2), 2 * (g - 2) + 1))
                            if g >= 3:
                                if g == 3:
                                    new_av(7, 0)
                                av_pairs(pts[7], avs[(7, 0)], 0,
                                         (2 * (g - 3), 2 * (g - 3) + 1))
                    den_pairs(pts[n], dens[n], (14, 15))

                # drain: finish block 7; ch0 tail on DVE, ch1 on gpsimd in
                # parallel, output DMAs on separate queues
                av_pairs(pts[7], avs[(7, 1)], 1, range(12, NPAIR))
                av_pairs(pts[7], avs[(7, 0)], 0, range(10, NPAIR))
                rgb7 = den_tail(7, dens[7])
                tmp0 = wrk.tile([128, IBLK], F32, name="tmp7_0", tag="tmp")
                nc.vector.tensor_tensor(tmp0[:], avs[(7, 0)][:], rgb7[:], MULT)
                ot0 = wrk.tile([128, IBLK], F32, name="ot7_0", tag="ot", bufs=3)
                nc.vector.scalar_tensor_tensor(
                    ot0[:], tmp0[:], gbv_sb[:, 0:1], xfs[7][0][:], ADD, ADD)
                nc.sync.dma_start(out[0:128, 7 * IBLK:8 * IBLK], ot0[:])
                tmp1 = wrk.tile([128, IBLK], F32, name="tmp7_1", tag="tmp")
                nc.gpsimd.scalar_tensor_tensor(
                    tmp1[:], avs[(7, 1)][:], 1.0, rgb7[:], MULT, MULT)
                ot1 = wrk.tile([128, IBLK], F32, name="ot7_1", tag="ot", bufs=3)
                nc.gpsimd.scalar_tensor_tensor(
                    ot1[:], tmp1[:], gbv_sb[:, 1:2], xfs[7][1][:], ADD, ADD)
                nc.gpsimd.dma_start(out[128:256, 7 * IBLK:8 * IBLK], ot1[:])
    nc.compile()
    return nc


_NC_CACHE = {}


def kernel(x, y, Wq, bq, Wk, bk, Wv, bv, gamma):
    assert x.shape == (B, C, 64, 64)
    xs = np.ascontiguousarray(x.reshape(B, C, HW)).astype(np.float32)
    ys = np.ascontiguousarray(y.reshape(B, C, HW)).astype(np.float32)

    # x: fp16, chunk-major [p, (g, h, 512)] for one-DMA-per-chunk loads
    x16 = (xs.reshape(B, 2, 128, NIB, IBLK).transpose(0, 2, 3, 1, 4)
           .reshape(B, 128, CH * HW).astype(np.float16))
    # y: fp8, [p, (g, h, s, 512)] with channel c = h*128 + s*64 + p
    y8 = (ys.reshape(B, 2, 2, 64, NIB, IBLK).transpose(0, 3, 4, 1, 2, 5)
          .reshape(B, 64, 4 * HW).astype(F8NP))
    wq16 = (Wq.T.reshape(2, 128, D).transpose(1, 0, 2)
            .reshape(128, CH * D).astype(np.float16))
    wk8 = (Wk.T.reshape(2, 2, 64, D).transpose(2, 0, 1, 3)
           .reshape(64, 4 * D).astype(F8NP))
    g = float(np.asarray(gamma).reshape(-1)[0])
    wv8 = ((g * Wv).T.reshape(2, 2, 64, C).transpose(2, 0, 1, 3)
           .reshape(64, 4 * C).astype(F8NP))
    bqr = np.ascontiguousarray(bq.reshape(1, D)).astype(BF16NP)
    bkr = np.ascontiguousarray(bk.reshape(1, D)).astype(BF16NP)
    gbvh = np.ascontiguousarray((g * bv.astype(np.float32)).reshape(CH, 128).T)

    if "nc" not in _NC_CACHE:
        _NC_CACHE["nc"] = _build()
    nc = _NC_CACHE["nc"]

    in_maps = [
        {
            "x16d": np.ascontiguousarray(x16[b]),
            "y8d": np.ascontiguousarray(y8[b]),
            "xfd": np.ascontiguousarray(xs[b]),
            "wq16d": np.ascontiguousarray(wq16),
            "wk8d": np.ascontiguousarray(wk8),
            "wv8d": np.ascontiguousarray(wv8),
            "bqrd": bqr, "bkrd": bkr, "gbvd": gbvh,
        }
        for b in range(B)
    ]
    res = run_bass_kernel_spmd(nc, in_maps, list(range(B)))
    outs = np.stack([res.results[b]["out"] for b in range(B)])
    return outs.reshape(B, C, 64, 64).astype(np.float32)
